# revision 1
# baseline (speedup 1.0000x reference)
"""Trainium2 Bass kernel for nn_Better_Transformer (block-diag MLP + BatchNorm + tanh ×2).

  o1 = tanh(BN(x @ blockdiag(w1) + b1))
  o3 = tanh(BN(o1 @ blockdiag(w2) + b2 + x))

Strategy (8 NeuronCores, data-parallel over the batch dim):
  - Each core owns 2048 of the 16384 rows; weights/BN params replicated.
  - Feature-major layout on chip ([128 features, rows]): BatchNorm
    reductions are free-dim reductions and matmuls stream rows as the
    moving operand (weights stationary), N=1024 bf16 moving tiles.
  - Host pre-transposes x to [F, B/8] bf16 per core; output returns
    feature-major bf16 and the host transposes/upcasts back.
  - bias1/bias2 cancel inside BatchNorm and never reach the device.
  - BN statistics: per-core (mean, E[y²]) per feature → 32 KB AllGather
    over the 8 cores → local reduce → global mean/var.  Stage-A stats
    are split between VectorE (bn_stats) and ScalarE (Copy/Square with
    accum_out) so both engines share the scan.
  - The residual (+x) is accumulated on the TensorEngine via an
    identity-matrix matmul into the same PSUM group as matmul2.
  - BN affine + tanh fuse into one ScalarEngine activation per tile
    (per-partition scale/bias APs).
  - y1 is recomputed in stage B instead of stored; u = o2+x overwrites
    the resident x blockwise (one 16 MB SBUF region holds x then u).
  - A warm-up burst of matmuls trips the PE HAM throttle to 2.4 GHz
    while the input DMAs are still in flight.
"""

import os
import sys
import types

import numpy as np
import ml_dtypes

B, F, P, D = 16384, 4096, 32, 128
NCORES = 8
BC = B // NCORES          # 2048 rows per core
NW = 1024                 # matmul moving-dim (bf16 allows 1024)
NH = BC // NW             # 2 wide chunks per block row-range
EPS = 1e-5

# Stage-A engine split: these blocks' stats run on ScalarE (accum_out),
# the rest on VectorE (bn_stats).  ~13/32 balances 2×FD1024 ACT ops
# against 4×FD512 bn_stats.
ACT_BLOCKS = [0, 3, 6, 9, 12, 15, 18, 21, 24, 27, 30]
DVE_BLOCKS = [p for p in range(P) if p not in ACT_BLOCKS]
# Sync-1 runs as two half-batch AllGathers (blocks 0-15 gathered while
# blocks 16-31 are still computing).  Payload column order groups by
# (half, engine) so every payload write is a contiguous batched op.
DVE_A = [p for p in DVE_BLOCKS if p < 16]
ACT_A = [p for p in ACT_BLOCKS if p < 16]
DVE_B = [p for p in DVE_BLOCKS if p >= 16]
ACT_B = [p for p in ACT_BLOCKS if p >= 16]
GROUPED = DVE_A + ACT_A + DVE_B + ACT_B
COL1 = {p: i for i, p in enumerate(GROUPED)}
NDA, NAA, NDB, NAB = len(DVE_A), len(ACT_A), len(DVE_B), len(ACT_B)

_BF16 = ml_dtypes.bfloat16

_state: dict = {}


def _install_ldw_opt_patch():
    """bass hardcodes --enable-ldw-opt=false; walrus's own default is
    true.  Re-enable it (BASS_LDW_OPT=0 reverts) so repeated-lhsT matmul
    runs don't reload the PE weight array every instruction."""
    if _state.get("ldw_patched") or os.environ.get("BASS_LDW_OPT", "0") != "1":
        return
    _state["ldw_patched"] = True
    import concourse.bass_utils as bu
    real = bu.run_command

    def wrapper(argv, **kw):
        argv = ["--enable-ldw-opt=true" if a == "--enable-ldw-opt=false" else a
                for a in argv]
        return real(argv, **kw)

    bu.run_command = wrapper


def _install_tile_drain_patch():
    """This walrus build rejects >1 sem wait per instruction ("Too many
    sync wait commands" in setupSyncWait).  1) split the end-of-kernel
    drain waits across single-wait NOPs; 2) after assign_waits, hoist
    extra per-instruction waits onto nofuse NOPs."""
    if _state.get("patched"):
        return
    _state["patched"] = True
    import concourse.mybir as mybir
    import concourse.tile as tile_mod
    from concourse.tile import TileContext
    from concourse.vector_clock import ScopedClock, VectorClock

    def _drain_and_barrier(self, tick_clock, wait_clock):
        gc = tick_clock.global_clock
        for i in range(len(gc)):
            if gc[i] > 0:
                c = VectorClock()
                c.require_at_least(i, gc[i])
                nop = self.nc.sync.nop(nofuse=True, hint="tile_exit_wait")
                wait_clock.add_sem_waits(nop.ins, ScopedClock({None: c}))
        self.nc.sync.drain()
        self.nc.all_engine_barrier()
        assert self.sems is not None
        popped = self.nc._tile_sem_poison_stack.pop()
        assert popped is self._sem_poison
        self.nc.clear_and_free_semaphores(list(self.sems.allocated().values()))
        self.nc.all_engine_barrier()

    TileContext._drain_and_barrier = _drain_and_barrier

    _RealWait = tile_mod.TileClockWait

    class _WaitSplitClockWait:
        def __init__(self, tc, ordered):
            self._w = _RealWait(tc, ordered)
            self._tc = tc
            self._ordered = ordered

        def assign_waits(self, bb_name):
            r = self._w.assign_waits(bb_name)
            nc = self._tc.nc
            for insts in self._ordered.values():
                out = []
                for inst in insts:
                    si = inst.sync_info
                    if si is not None and si.on_wait and len(si.on_wait) > 1:
                        waits = list(si.on_wait)
                        for w in waits[:-1]:
                            nop = mybir.InstNoOp(
                                name=nc.get_next_instruction_name(),
                                engine=inst.engine, ins=[], outs=[],
                            )
                            nop.bass_nofuse = True
                            nop.sync_info = mybir.SyncInfo(on_wait=[w], on_update=[])
                            out.append(nop)
                        si.on_wait = [waits[-1]]
                    out.append(inst)
                insts[:] = out
            return r

        def __getattr__(self, k):
            return getattr(self._w, k)

    tile_mod.TileClockWait = _WaitSplitClockWait


def _install_ntff_hook():
    """Optional: lets BASS_TRACE=1 produce an NTFF profile under axon when
    the image's antenv lacks axon_hooks.  Safe no-op on any failure."""
    if "antenv.axon_hooks" in sys.modules:
        return
    try:
        import contextlib
        import ctypes

        so_path = "/opt/axon/libaxon_pjrt.so"
        if not os.path.exists(so_path):
            return
        lib = ctypes.CDLL(so_path)
        if not hasattr(lib, "axon_start_nrt_profile"):
            return
        lib.axon_start_nrt_profile.argtypes = [ctypes.POINTER(ctypes.c_int64), ctypes.c_size_t]
        lib.axon_start_nrt_profile.restype = ctypes.c_int64
        lib.axon_stop_nrt_profile.argtypes = [ctypes.c_char_p]
        lib.axon_stop_nrt_profile.restype = ctypes.c_int64

        @contextlib.contextmanager
        def _hook(output_dir, device_ids):
            import jax
            jax.devices()
            if device_ids:
                ids = (ctypes.c_int64 * len(device_ids))(*device_ids)
                rc = lib.axon_start_nrt_profile(ids, len(device_ids))
            else:
                rc = lib.axon_start_nrt_profile(None, 0)
            if rc != 0:
                raise RuntimeError(f"axon_start_nrt_profile rc={rc}")
            try:
                yield
            finally:
                n = lib.axon_stop_nrt_profile(str(output_dir).encode())
                if n <= 0:
                    print(f"ntff profile: {n} files written", file=sys.stderr)

        mod = types.ModuleType("antenv.axon_hooks")
        mod.get_axon_ntff_profile_hook = lambda: _hook
        mod.set_axon_ntff_profile_hook = lambda h: None
        sys.modules["antenv.axon_hooks"] = mod
    except Exception:
        pass


def _build():
    import concourse.bass as bass
    import concourse.mybir as mybir
    import concourse.tile as tile

    f32 = mybir.dt.float32
    bf16 = mybir.dt.bfloat16
    Tanh = mybir.ActivationFunctionType.Tanh
    Sqrt = mybir.ActivationFunctionType.Sqrt
    Copy = mybir.ActivationFunctionType.Copy
    Square = mybir.ActivationFunctionType.Square
    mult = mybir.AluOpType.mult
    add = mybir.AluOpType.add
    subtract = mybir.AluOpType.subtract
    AX = mybir.AxisListType.X

    nc = bass.Bass(trn_type="TRN2", num_devices=NCORES)

    xt = nc.dram_tensor("xt", [F, BC], bf16, kind="ExternalInput")
    w1 = nc.dram_tensor("w1", [D, F], bf16, kind="ExternalInput")
    w2 = nc.dram_tensor("w2", [D, F], bf16, kind="ExternalInput")
    ident = nc.dram_tensor("ident", [D, D], bf16, kind="ExternalInput")
    g1 = nc.dram_tensor("g1", [D, P], f32, kind="ExternalInput")   # grouped col order
    bt1 = nc.dram_tensor("bt1", [D, P], f32, kind="ExternalInput")  # grouped col order
    g3 = nc.dram_tensor("g3", [D, P], f32, kind="ExternalInput")   # natural order
    bt3 = nc.dram_tensor("bt3", [D, P], f32, kind="ExternalInput")
    out = nc.dram_tensor("out", [F, BC], bf16, kind="ExternalOutput")

    n_act = len(ACT_BLOCKS)
    n_dve = len(DVE_BLOCKS)

    with tile.TileContext(nc) as tc:
        with (
            tc.tile_pool(name="const", bufs=1) as const,
            tc.tile_pool(name="xup", bufs=1) as xup,
            tc.tile_pool(name="stat", bufs=1) as statp,
            tc.tile_pool(name="o1p", bufs=2) as o1p,
            tc.tile_pool(name="scrp", bufs=2) as scrp,
            tc.tile_pool(name="ofp", bufs=4) as ofp,
            tc.tile_pool(name="psa", bufs=2, space="PSUM") as psa,
            tc.tile_pool(name="psb", bufs=2, space="PSUM") as psb,
            tc.tile_pool(name="dram", bufs=1, space="DRAM") as dram,
        ):
            w1_sb = const.tile([D, F], bf16)
            w2_sb = const.tile([D, F], bf16)
            id_sb = const.tile([D, D], bf16)
            g1_sb = const.tile([D, P], f32)
            bt1_sb = const.tile([D, P], f32)
            g3_sb = const.tile([D, P], f32)
            bt3_sb = const.tile([D, P], f32)
            nc.sync.dma_start(w1_sb, w1[:])
            nc.sync.dma_start(w2_sb, w2[:])
            nc.sync.dma_start(id_sb, ident[:])
            nc.sync.dma_start(g1_sb, g1[:])
            nc.sync.dma_start(bt1_sb, bt1[:])
            nc.sync.dma_start(g3_sb, g3[:])
            nc.sync.dma_start(bt3_sb, bt3[:])

            # PE HAM warm-up: a dense burst of matmuls on the (tiny) w1
            # tile while the big xt DMAs stream in.
            for i in range(24):
                pw = psa.tile([D, NW], f32, tag="pp")
                nc.tensor.matmul(pw[:, 0:NW // 2], lhsT=w1_sb[:, 0:D],
                                 rhs=w1_sb[:, 0:NW // 2], start=True, stop=True)
                nc.tensor.matmul(pw[:, NW // 2:NW], lhsT=w1_sb[:, 0:D],
                                 rhs=w1_sb[:, NW // 2:NW], start=True, stop=True)

            xu = []
            for p in range(P):
                t = xup.tile([D, BC], bf16, tag=f"xu{p}")
                nc.sync.dma_start(t, xt[p * D:(p + 1) * D, :])
                xu.append(t)

            stats1 = statp.tile([D, n_dve, 4, 6], f32)   # DVE blocks, 512-wide
            stats2 = statp.tile([D, P, 4, 6], f32)
            mv1 = statp.tile([D, n_dve, 2], f32)
            mv2 = statp.tile([D, P, 2], f32)
            sa = statp.tile([D, n_act, 2], f32)          # ACT-block sums
            qa = statp.tile([D, n_act, 2], f32)          # ACT-block sumsqs
            arpay1a = statp.tile([D, P], f32)
            arpay1b = statp.tile([D, P], f32)
            arpay2q = [statp.tile([D, 16], f32, name=f"arpay2q{q}") for q in range(4)]
            red1a = statp.tile([D, P], f32)
            red1b = statp.tile([D, P], f32)
            red2q = [statp.tile([D, 16], f32, name=f"red2q{q}") for q in range(4)]
            gath1a = statp.tile([D, NCORES, P], f32)
            gath1b = statp.tile([D, NCORES, P], f32)
            gath2q = [statp.tile([D, NCORES, 16], f32, name=f"gath2q{q}")
                      for q in range(4)]
            Mt = statp.tile([D, P], f32)
            Qt = statp.tile([D, P], f32)
            vt = statp.tile([D, P], f32)
            s1 = statp.tile([D, P], f32)
            t1 = statp.tile([D, P], f32)
            s3 = statp.tile([D, P], f32)
            t3 = statp.tile([D, P], f32)
            eps_sb = statp.tile([D, 1], f32)
            nc.vector.memset(eps_sb, EPS)

            def wcol(w_sb, p):
                return w_sb[:, p * D:(p + 1) * D]

            def all_gather(arpay, gath, red, tagn):
                npay = arpay.shape[-1]
                agin = dram.tile([D, npay], f32, tag=f"agin{tagn}", name=f"agin{tagn}")
                agout = dram.tile([NCORES * D, npay], f32, tag=f"agout{tagn}",
                                  name=f"agout{tagn}")
                nc.sync.dma_start(agin, arpay)
                nc.gpsimd.collective_compute(
                    "AllGather", mybir.AluOpType.bypass,
                    replica_groups=[list(range(NCORES))],
                    ins=[agin.opt()], outs=[agout.opt()],
                )
                nc.sync.dma_start(gath, agout.rearrange("(r i) f -> i r f", r=NCORES))
                nc.vector.tensor_reduce(out=red, in_=gath[:].rearrange("i r f -> i f r"),
                                        axis=AX, op=add)

            def affine(red, g_sb, b_sb, s, t):
                # red[:, 0:P] = Σ_cores mean ; red[:, P:2P] = Σ_cores E[y²]
                nc.vector.tensor_scalar_mul(Mt, red[:, 0:P], 1.0 / NCORES)
                nc.vector.tensor_scalar_mul(Qt, red[:, P:2 * P], 1.0 / NCORES)
                nc.vector.tensor_tensor(vt, Mt, Mt, op=mult)
                nc.vector.tensor_tensor(vt, Qt, vt, op=subtract)          # global var
                nc.scalar.activation(out=vt, in_=vt, func=Sqrt, bias=eps_sb)
                nc.vector.reciprocal(vt, vt)                              # rstd
                nc.vector.tensor_tensor(s, g_sb, vt, op=mult)
                nc.vector.tensor_tensor(t, Mt, s, op=mult)
                nc.vector.tensor_tensor(t, b_sb, t, op=subtract)          # beta - M*s

            # ---- Stage A: per-core stats of y1 = x @ W1 ----
            for p in range(P):
                j = None
                if p in ACT_BLOCKS:
                    j = ACT_BLOCKS.index(p)
                else:
                    j = DVE_BLOCKS.index(p)
                pool = psa if p % 2 == 0 else psb
                for h in range(NH):
                    ps = pool.tile([D, NW], f32, tag="pp" if pool is psa else "qq")
                    for q in range(2):
                        qs = slice(q * (NW // 2), (q + 1) * (NW // 2))
                        nc.tensor.matmul(ps[:, qs], lhsT=wcol(w1_sb, p),
                                         rhs=xu[p][:, h * NW + q * (NW // 2):
                                                   h * NW + (q + 1) * (NW // 2)],
                                         start=True, stop=True)
                    if p in ACT_BLOCKS:
                        scr = scrp.tile([D, NW], bf16, tag="scr")
                        nc.scalar.activation(out=scr, in_=ps, func=Copy,
                                             accum_out=sa[:, j, h:h + 1])
                        nc.scalar.activation(out=scr, in_=ps, func=Square,
                                             accum_out=qa[:, j, h:h + 1])
                    else:
                        nc.vector.bn_stats(out=stats1[:, j, 2 * h], in_=ps[:, 0:NW // 2])
                        nc.vector.bn_stats(out=stats1[:, j, 2 * h + 1], in_=ps[:, NW // 2:NW])
                if p not in ACT_BLOCKS:
                    nc.vector.bn_aggr(out=mv1[:, j], in_=stats1[:, j])

                if p == 15:
                    # half-a payload: [DVE_A means | ACT_A means | DVE_A E2 | ACT_A E2]
                    h2 = P // 2
                    nc.vector.tensor_copy(arpay1a[:, 0:NDA], mv1[:, 0:NDA, 0])
                    nc.vector.tensor_tensor(arpay1a[:, h2:h2 + NDA], mv1[:, 0:NDA, 0],
                                            mv1[:, 0:NDA, 0], op=mult)
                    nc.vector.tensor_tensor(arpay1a[:, h2:h2 + NDA],
                                            arpay1a[:, h2:h2 + NDA],
                                            mv1[:, 0:NDA, 1], op=add)
                    nc.vector.tensor_reduce(out=arpay1a[:, NDA:h2],
                                            in_=sa[:, 0:NAA], axis=AX, op=add)
                    nc.vector.tensor_reduce(out=arpay1a[:, h2 + NDA:P],
                                            in_=qa[:, 0:NAA], axis=AX, op=add)
                    nc.vector.tensor_scalar_mul(arpay1a[:, NDA:h2],
                                                arpay1a[:, NDA:h2], 1.0 / BC)
                    nc.vector.tensor_scalar_mul(arpay1a[:, h2 + NDA:P],
                                                arpay1a[:, h2 + NDA:P], 1.0 / BC)
                    all_gather(arpay1a, gath1a, red1a, "1a")

            # half-b payload
            h2 = P // 2
            nc.vector.tensor_copy(arpay1b[:, 0:NDB], mv1[:, NDA:n_dve, 0])
            nc.vector.tensor_tensor(arpay1b[:, h2:h2 + NDB], mv1[:, NDA:n_dve, 0],
                                    mv1[:, NDA:n_dve, 0], op=mult)
            nc.vector.tensor_tensor(arpay1b[:, h2:h2 + NDB], arpay1b[:, h2:h2 + NDB],
                                    mv1[:, NDA:n_dve, 1], op=add)
            nc.vector.tensor_reduce(out=arpay1b[:, NDB:h2], in_=sa[:, NAA:n_act],
                                    axis=AX, op=add)
            nc.vector.tensor_reduce(out=arpay1b[:, h2 + NDB:P], in_=qa[:, NAA:n_act],
                                    axis=AX, op=add)
            nc.vector.tensor_scalar_mul(arpay1b[:, NDB:h2], arpay1b[:, NDB:h2], 1.0 / BC)
            nc.vector.tensor_scalar_mul(arpay1b[:, h2 + NDB:P],
                                        arpay1b[:, h2 + NDB:P], 1.0 / BC)
            all_gather(arpay1b, gath1b, red1b, "1b")

            # keep the PE HAM warm through the collective gap (slot reuse of
            # the "pp" pool orders these after stage A's matmuls)
            for i in range(20):
                pw = psa.tile([D, NW], f32, tag="pp", name="pw")
                nc.tensor.matmul(pw[:, 0:NW // 2], lhsT=w1_sb[:, 0:D],
                                 rhs=w1_sb[:, 0:NW // 2], start=True, stop=True)
                nc.tensor.matmul(pw[:, NW // 2:NW], lhsT=w1_sb[:, 0:D],
                                 rhs=w1_sb[:, NW // 2:NW], start=True, stop=True)

            # affine from the two half-gathers (col order = GROUPED)
            nc.vector.tensor_scalar_mul(Mt[:, 0:h2], red1a[:, 0:h2], 1.0 / NCORES)
            nc.vector.tensor_scalar_mul(Mt[:, h2:P], red1b[:, 0:h2], 1.0 / NCORES)
            nc.vector.tensor_scalar_mul(Qt[:, 0:h2], red1a[:, h2:P], 1.0 / NCORES)
            nc.vector.tensor_scalar_mul(Qt[:, h2:P], red1b[:, h2:P], 1.0 / NCORES)
            nc.vector.tensor_tensor(vt, Mt, Mt, op=mult)
            nc.vector.tensor_tensor(vt, Qt, vt, op=subtract)
            nc.scalar.activation(out=vt, in_=vt, func=Sqrt, bias=eps_sb)
            nc.vector.reciprocal(vt, vt)
            nc.vector.tensor_tensor(s1, g1_sb, vt, op=mult)
            nc.vector.tensor_tensor(t1, Mt, s1, op=mult)
            nc.vector.tensor_tensor(t1, bt1_sb, t1, op=subtract)

            # ---- Stage B: o1 = tanh(s1·y1 + t1); u = o1 @ W2 + x ----
            for p in range(P):
                c1 = COL1[p]
                o1 = o1p.tile([D, BC], bf16, tag="o1")
                pss = []
                for h in range(NH):
                    ps = psa.tile([D, NW], f32, tag="pp")
                    pss.append(ps)
                    for q in range(2):
                        nc.tensor.matmul(ps[:, q * (NW // 2):(q + 1) * (NW // 2)],
                                         lhsT=wcol(w1_sb, p),
                                         rhs=xu[p][:, h * NW + q * (NW // 2):
                                                   h * NW + (q + 1) * (NW // 2)],
                                         start=True, stop=True)
                for h in range(NH):
                    hs = slice(h * NW, (h + 1) * NW)
                    nc.scalar.activation(out=o1[:, hs], in_=pss[h], func=Tanh,
                                         bias=t1[:, c1:c1 + 1], scale=s1[:, c1:c1 + 1])
                # one LDW of W2 for all four halves, then one LDW of identity
                pus = [psb.tile([D, NW], f32, tag="qq", name=f"pu{h}") for h in range(NH)]
                for h in range(NH):
                    for q in range(2):
                        gsl = slice(h * NW + q * (NW // 2), h * NW + (q + 1) * (NW // 2))
                        nc.tensor.matmul(pus[h][:, q * (NW // 2):(q + 1) * (NW // 2)],
                                         lhsT=wcol(w2_sb, p), rhs=o1[:, gsl],
                                         start=True, stop=False)
                for h in range(NH):
                    for q in range(2):
                        gsl = slice(h * NW + q * (NW // 2), h * NW + (q + 1) * (NW // 2))
                        nc.tensor.matmul(pus[h][:, q * (NW // 2):(q + 1) * (NW // 2)],
                                         lhsT=id_sb, rhs=xu[p][:, gsl],
                                         start=False, stop=True)
                for h in range(NH):
                    hs = slice(h * NW, (h + 1) * NW)
                    if p < 10:
                        nc.scalar.activation(out=xu[p][:, hs], in_=pus[h],
                                             func=Copy)   # u overwrites x
                    else:
                        nc.vector.tensor_copy(out=xu[p][:, hs], in_=pus[h])
                    nc.vector.bn_stats(out=stats2[:, p, 2 * h],
                                       in_=xu[p][:, h * NW:h * NW + NW // 2])
                    nc.vector.bn_stats(out=stats2[:, p, 2 * h + 1],
                                       in_=xu[p][:, h * NW + NW // 2:(h + 1) * NW])
                nc.vector.bn_aggr(out=mv2[:, p], in_=stats2[:, p])

                if p % 8 == 7:
                    q = p // 8
                    lo = q * 8
                    nc.vector.tensor_copy(arpay2q[q][:, 0:8], mv2[:, lo:lo + 8, 0])
                    nc.vector.tensor_tensor(arpay2q[q][:, 8:16], mv2[:, lo:lo + 8, 0],
                                            mv2[:, lo:lo + 8, 0], op=mult)
                    nc.vector.tensor_tensor(arpay2q[q][:, 8:16], arpay2q[q][:, 8:16],
                                            mv2[:, lo:lo + 8, 1], op=add)
                    all_gather(arpay2q[q], gath2q[q], red2q[q], f"2q{q}")

            def affine2(red, lo, hi):
                w = hi - lo
                nc.vector.tensor_scalar_mul(Mt[:, lo:hi], red[:, 0:w], 1.0 / NCORES)
                nc.vector.tensor_scalar_mul(Qt[:, lo:hi], red[:, w:2 * w], 1.0 / NCORES)
                nc.vector.tensor_tensor(vt[:, lo:hi], Mt[:, lo:hi], Mt[:, lo:hi], op=mult)
                nc.vector.tensor_tensor(vt[:, lo:hi], Qt[:, lo:hi], vt[:, lo:hi],
                                        op=subtract)
                nc.scalar.activation(out=vt[:, lo:hi], in_=vt[:, lo:hi], func=Sqrt,
                                     bias=eps_sb)
                nc.vector.reciprocal(vt[:, lo:hi], vt[:, lo:hi])
                nc.vector.tensor_tensor(s3[:, lo:hi], g3_sb[:, lo:hi], vt[:, lo:hi],
                                        op=mult)
                nc.vector.tensor_tensor(t3[:, lo:hi], Mt[:, lo:hi], s3[:, lo:hi], op=mult)
                nc.vector.tensor_tensor(t3[:, lo:hi], bt3_sb[:, lo:hi], t3[:, lo:hi],
                                        op=subtract)

            # ---- Stage C: out = tanh(s3·u + t3), flowing in per sync-2 quarter ----
            for q in range(4):
                affine2(red2q[q], q * 8, q * 8 + 8)
                for p in range(q * 8, q * 8 + 8):
                    of = ofp.tile([D, BC], bf16, tag="of", name="of")
                    nc.scalar.activation(out=of, in_=xu[p], func=Tanh,
                                         bias=t3[:, p:p + 1], scale=s3[:, p:p + 1])
                    nc.sync.dma_start(out[p * D:(p + 1) * D, :], of)

    return nc


def _get_nc():
    if "nc" not in _state:
        _install_tile_drain_patch()
        _install_ldw_opt_patch()
        _install_ntff_hook()
        _state["nc"] = _build()
    return _state["nc"]


def kernel(x, weights1, bias1, weights2, bias2, gamma1, beta1, gamma3, beta3):
    from concourse.bass_utils import run_bass_kernel_spmd

    x = np.asarray(x, dtype=np.float32)
    w1 = np.asarray(weights1, dtype=np.float32)
    w2 = np.asarray(weights2, dtype=np.float32)
    gamma1 = np.asarray(gamma1, dtype=np.float32)
    beta1 = np.asarray(beta1, dtype=np.float32)
    gamma3 = np.asarray(gamma3, dtype=np.float32)
    beta3 = np.asarray(beta3, dtype=np.float32)

    nc = _get_nc()

    xT = np.ascontiguousarray(x.T).astype(_BF16)            # [F, B]
    w1h = np.ascontiguousarray(w1.transpose(1, 0, 2).reshape(D, F)).astype(_BF16)
    w2h = np.ascontiguousarray(w2.transpose(1, 0, 2).reshape(D, F)).astype(_BF16)
    identh = np.eye(D, dtype=np.float32).astype(_BF16)
    perm = np.asarray(GROUPED)
    g1h = np.ascontiguousarray(gamma1.reshape(P, D).T[:, perm])
    bt1h = np.ascontiguousarray(beta1.reshape(P, D).T[:, perm])
    g3h = np.ascontiguousarray(gamma3.reshape(P, D).T)
    bt3h = np.ascontiguousarray(beta3.reshape(P, D).T)

    in_maps = []
    for cid in range(NCORES):
        in_maps.append({
            "xt": np.ascontiguousarray(xT[:, cid * BC:(cid + 1) * BC]),
            "w1": w1h, "w2": w2h, "ident": identh,
            "g1": g1h, "bt1": bt1h, "g3": g3h, "bt3": bt3h,
        })

    res = run_bass_kernel_spmd(nc, in_maps, core_ids=list(range(NCORES)))
    _state["last_exec_time_ns"] = res.exec_time_ns

    outT = np.empty((B, F), dtype=np.float32)
    for cid in range(NCORES):
        outT[cid * BC:(cid + 1) * BC, :] = res.results[cid]["out"].T.astype(np.float32)
    return outT



# revision 3
# speedup vs baseline: 1.1590x; 1.1590x over previous
"""Trainium2 Bass kernel for nn_Better_Transformer (block-diag MLP + BatchNorm + tanh ×2).

  o1 = tanh(BN(x @ blockdiag(w1) + b1))
  o3 = tanh(BN(o1 @ blockdiag(w2) + b2 + x))

Strategy (8 NeuronCores, FEATURE-sharded — zero collectives):
  The network is fully block-diagonal per 128-feature block: output block p
  depends only on input block p (block matmuls are per-block, the residual is
  elementwise, BatchNorm normalizes over the batch dim).  Each core owns 4 of
  the 32 blocks (512 features) and ALL 16384 rows, so BN statistics are fully
  core-local and both AllReduce sync points of the data-parallel layout
  disappear.

  Per core, feature-major layout ([128 features, 16384 rows] bf16 per block):
  - Stage A (per block): matmul1 chunks -> PSUM, bn_stats on VectorE.
    bias1/bias2 cancel inside BatchNorm and never reach the device.
  - rstd via Newton iteration on VectorE (no Sqrt on ScalarE -> the ACT
    engine runs a single act-table set {Tanh, Copy} with zero reloads).
  - Stage B (per block): matmul1 recomputed (cheaper than storing y1),
    tanh1 on ScalarE (PSUM->SBUF bf16), matmul2, then u = o2 + x:
    ~11/16 of chunks add x via an identity matmul on the TensorEngine and
    copy PSUM->SBUF on ScalarE (Copy + accum_out gives sum(u) free);
    ~5/16 of chunks use a fused scalar_tensor_tensor on VectorE
    (add + copy + sum in one op).  u overwrites x in place.
    sum(u^2) via tensor_tensor square (2x bf16 mode) + tensor_scalar accum
    (4x mode) on VectorE.
  - Stage C (per block): tanh2 on ScalarE (SBUF->SBUF), DMA out.
  - Blocks are software-pipelined (A0 A1 B0 A2 B1 C0 A3 B2 C1 B3 C2 C3) so
    each engine's in-order queue never convoys on another engine.
  - A warm-up burst of matmuls trips the PE HAM throttle up while the input
    DMAs are still in flight.
"""

import os
import sys
import types

import numpy as np
import ml_dtypes

B, F, P, D = 16384, 4096, 32, 128
NCORES = 8
NBLK = P // NCORES        # 4 blocks per core
FC = NBLK * D             # 512 features per core
R = B                     # all 16384 rows on every core
CH = 512                  # stage-A chunk / matmul moving width
NCA = R // CH             # 32 stats chunks per block
UCH = 1024                # u-production chunk
NCU = R // UCH            # 16 u chunks per block
SQCH = 2048               # squared-sum chunk
NSQ = R // SQCH           # 8
TCH = 4096                # tanh2 chunk
NTC = R // TCH            # 4
EPS = 1e-5

# u-production engine split: chunks in DVE_U run on VectorE (fused STT),
# the rest on ScalarE (identity-matmul residual + Copy w/ accum).
DVE_U = (0, 3, 6, 9, 12)
ACT_U = tuple(h for h in range(NCU) if h not in DVE_U)

NEWTON_ITERS = 5
SEED1 = 1.7               # 1/sqrt(var(y1)+eps), var(y1) ~ 1/3
SEED2 = 0.9               # 1/sqrt(var(u)+eps),  var(u)  ~ 1.2

_BF16 = ml_dtypes.bfloat16

_state: dict = {}


def _install_ldw_opt_patch():
    """bass hardcodes --enable-ldw-opt=false; walrus's own default is
    true.  Re-enable it (BASS_LDW_OPT=0 reverts) so repeated-lhsT matmul
    runs don't reload the PE weight array every instruction."""
    if _state.get("ldw_patched") or os.environ.get("BASS_LDW_OPT", "0") != "1":
        return
    _state["ldw_patched"] = True
    import concourse.bass_utils as bu
    real = bu.run_command

    def wrapper(argv, **kw):
        argv = ["--enable-ldw-opt=true" if a == "--enable-ldw-opt=false" else a
                for a in argv]
        return real(argv, **kw)

    bu.run_command = wrapper


def _install_tile_drain_patch():
    """This walrus build rejects >1 sem wait per instruction ("Too many
    sync wait commands" in setupSyncWait).  1) split the end-of-kernel
    drain waits across single-wait NOPs; 2) after assign_waits, hoist
    extra per-instruction waits onto nofuse NOPs."""
    if _state.get("patched"):
        return
    _state["patched"] = True
    import concourse.mybir as mybir
    import concourse.tile as tile_mod
    from concourse.tile import TileContext
    from concourse.vector_clock import ScopedClock, VectorClock

    def _drain_and_barrier(self, tick_clock, wait_clock):
        gc = tick_clock.global_clock
        for i in range(len(gc)):
            if gc[i] > 0:
                c = VectorClock()
                c.require_at_least(i, gc[i])
                nop = self.nc.sync.nop(nofuse=True, hint="tile_exit_wait")
                wait_clock.add_sem_waits(nop.ins, ScopedClock({None: c}))
        self.nc.sync.drain()
        self.nc.all_engine_barrier()
        assert self.sems is not None
        popped = self.nc._tile_sem_poison_stack.pop()
        assert popped is self._sem_poison
        self.nc.clear_and_free_semaphores(list(self.sems.allocated().values()))
        self.nc.all_engine_barrier()

    TileContext._drain_and_barrier = _drain_and_barrier

    _RealWait = tile_mod.TileClockWait

    class _WaitSplitClockWait:
        def __init__(self, tc, ordered):
            self._w = _RealWait(tc, ordered)
            self._tc = tc
            self._ordered = ordered

        def assign_waits(self, bb_name):
            r = self._w.assign_waits(bb_name)
            nc = self._tc.nc
            for insts in self._ordered.values():
                out = []
                for inst in insts:
                    si = inst.sync_info
                    if si is not None and si.on_wait and len(si.on_wait) > 1:
                        waits = list(si.on_wait)
                        for w in waits[:-1]:
                            nop = mybir.InstNoOp(
                                name=nc.get_next_instruction_name(),
                                engine=inst.engine, ins=[], outs=[],
                            )
                            nop.bass_nofuse = True
                            nop.sync_info = mybir.SyncInfo(on_wait=[w], on_update=[])
                            out.append(nop)
                        si.on_wait = [waits[-1]]
                    out.append(inst)
                insts[:] = out
            return r

        def __getattr__(self, k):
            return getattr(self._w, k)

    tile_mod.TileClockWait = _WaitSplitClockWait


def _install_ntff_hook():
    """Optional: lets BASS_TRACE=1 produce an NTFF profile under axon when
    the image's antenv lacks axon_hooks.  Safe no-op on any failure."""
    if "antenv.axon_hooks" in sys.modules:
        return
    try:
        import contextlib
        import ctypes

        so_path = "/opt/axon/libaxon_pjrt.so"
        if not os.path.exists(so_path):
            return
        lib = ctypes.CDLL(so_path)
        if not hasattr(lib, "axon_start_nrt_profile"):
            return
        lib.axon_start_nrt_profile.argtypes = [ctypes.POINTER(ctypes.c_int64), ctypes.c_size_t]
        lib.axon_start_nrt_profile.restype = ctypes.c_int64
        lib.axon_stop_nrt_profile.argtypes = [ctypes.c_char_p]
        lib.axon_stop_nrt_profile.restype = ctypes.c_int64

        @contextlib.contextmanager
        def _hook(output_dir, device_ids):
            import jax
            jax.devices()
            if device_ids:
                ids = (ctypes.c_int64 * len(device_ids))(*device_ids)
                rc = lib.axon_start_nrt_profile(ids, len(device_ids))
            else:
                rc = lib.axon_start_nrt_profile(None, 0)
            if rc != 0:
                raise RuntimeError(f"axon_start_nrt_profile rc={rc}")
            try:
                yield
            finally:
                n = lib.axon_stop_nrt_profile(str(output_dir).encode())
                if n <= 0:
                    print(f"ntff profile: {n} files written", file=sys.stderr)

        mod = types.ModuleType("antenv.axon_hooks")
        mod.get_axon_ntff_profile_hook = lambda: _hook
        mod.set_axon_ntff_profile_hook = lambda h: None
        sys.modules["antenv.axon_hooks"] = mod
    except Exception:
        pass


def _build():
    import concourse.bass as bass
    import concourse.mybir as mybir
    import concourse.tile as tile

    f32 = mybir.dt.float32
    bf16 = mybir.dt.bfloat16
    Tanh = mybir.ActivationFunctionType.Tanh
    Copy = mybir.ActivationFunctionType.Copy
    mult = mybir.AluOpType.mult
    add = mybir.AluOpType.add
    subtract = mybir.AluOpType.subtract
    AX = mybir.AxisListType.X

    nc = bass.Bass(trn_type="TRN2", num_devices=NCORES)

    xt = nc.dram_tensor("xt", [FC, R], bf16, kind="ExternalInput")
    w1 = nc.dram_tensor("w1", [D, FC], bf16, kind="ExternalInput")
    w2 = nc.dram_tensor("w2", [D, FC], bf16, kind="ExternalInput")
    ident = nc.dram_tensor("ident", [D, D], bf16, kind="ExternalInput")
    g1 = nc.dram_tensor("g1", [D, NBLK], f32, kind="ExternalInput")
    bt1 = nc.dram_tensor("bt1", [D, NBLK], f32, kind="ExternalInput")
    g3 = nc.dram_tensor("g3", [D, NBLK], f32, kind="ExternalInput")
    bt3 = nc.dram_tensor("bt3", [D, NBLK], f32, kind="ExternalInput")
    out = nc.dram_tensor("out", [FC, R], bf16, kind="ExternalOutput")

    with tile.TileContext(nc) as tc:
        with (
            tc.tile_pool(name="const", bufs=1) as const,
            tc.tile_pool(name="xup", bufs=1) as xup,
            tc.tile_pool(name="stat", bufs=1) as statp,
            tc.tile_pool(name="o1p", bufs=2) as o1p,
            tc.tile_pool(name="scp", bufs=2) as scp,
            tc.tile_pool(name="obp", bufs=2) as obp,
            tc.tile_pool(name="psa", bufs=2, space="PSUM") as psa,
            tc.tile_pool(name="psb", bufs=2, space="PSUM") as psb,
            tc.tile_pool(name="psc", bufs=2, space="PSUM") as psc,
        ):
            w1_sb = const.tile([D, FC], bf16)
            w2_sb = const.tile([D, FC], bf16)
            id_sb = const.tile([D, D], bf16)
            g1_sb = const.tile([D, NBLK], f32)
            bt1_sb = const.tile([D, NBLK], f32)
            g3_sb = const.tile([D, NBLK], f32)
            bt3_sb = const.tile([D, NBLK], f32)
            nc.sync.dma_start(w1_sb, w1[:])
            nc.sync.dma_start(w2_sb, w2[:])
            nc.sync.dma_start(id_sb, ident[:])
            nc.sync.dma_start(g1_sb, g1[:])
            nc.sync.dma_start(bt1_sb, bt1[:])
            nc.sync.dma_start(g3_sb, g3[:])
            nc.sync.dma_start(bt3_sb, bt3[:])

            # PE HAM warm-up while the x DMAs stream in.
            for _ in range(24):
                pw = psa.tile([D, CH], f32, tag="A")
                nc.tensor.matmul(pw, lhsT=w1_sb[:, 0:D], rhs=w1_sb[:, 0:CH],
                                 start=True, stop=True)

            xu = []
            for p in range(NBLK):
                t = xup.tile([D, R], bf16, tag=f"xu{p}")
                for q in range(4):
                    nc.sync.dma_start(t[:, q * (R // 4):(q + 1) * (R // 4)],
                                      xt[p * D:(p + 1) * D,
                                         q * (R // 4):(q + 1) * (R // 4)])
                xu.append(t)

            # per-block stat/state tiles (static: blocks are pipelined)
            st1 = [statp.tile([D, NCA, 6], f32, name=f"st1_{p}") for p in range(NBLK)]
            mv = [statp.tile([D, 2], f32, name=f"mv_{p}") for p in range(NBLK)]
            sumB = [statp.tile([D, NCU], f32, name=f"sumB_{p}") for p in range(NBLK)]
            sqB = [statp.tile([D, NSQ], f32, name=f"sqB_{p}") for p in range(NBLK)]
            s1 = [statp.tile([D, 1], f32, name=f"s1_{p}") for p in range(NBLK)]
            t1 = [statp.tile([D, 1], f32, name=f"t1_{p}") for p in range(NBLK)]
            s3 = [statp.tile([D, 1], f32, name=f"s3_{p}") for p in range(NBLK)]
            t3 = [statp.tile([D, 1], f32, name=f"t3_{p}") for p in range(NBLK)]
            za = [statp.tile([D, 1], f32, name=f"za_{p}") for p in range(NBLK)]
            ya = [statp.tile([D, 1], f32, name=f"ya_{p}") for p in range(NBLK)]
            ta = [statp.tile([D, 1], f32, name=f"ta_{p}") for p in range(NBLK)]
            ms = [statp.tile([D, 2], f32, name=f"ms_{p}") for p in range(NBLK)]

            def wcol(w_sb, p):
                return w_sb[:, p * D:(p + 1) * D]

            def newton_rsqrt(y, z, tmp, seed):
                # y := 1/sqrt(z), z > 0
                nc.vector.memset(y, seed)
                for _ in range(NEWTON_ITERS):
                    nc.vector.scalar_tensor_tensor(
                        out=tmp, in0=y, scalar=y, in1=z, op0=mult, op1=mult)
                    nc.vector.tensor_scalar(
                        out=tmp, in0=tmp, scalar1=-0.5, scalar2=1.5,
                        op0=mult, op1=add)
                    nc.vector.tensor_scalar(
                        out=y, in0=y, scalar1=tmp, scalar2=None, op0=mult)

            def emit_A(p):
                for h in range(NCA):
                    ps = psa.tile([D, CH], f32, tag="A")
                    nc.tensor.matmul(ps, lhsT=wcol(w1_sb, p),
                                     rhs=xu[p][:, h * CH:(h + 1) * CH],
                                     start=True, stop=True)
                    nc.vector.bn_stats(out=st1[p][:, h], in_=ps)
                nc.vector.bn_aggr(out=mv[p], in_=st1[p])
                # affine1: s1 = g1 * rstd ; t1 = b1 - mean * s1
                nc.vector.tensor_scalar(out=za[p], in0=mv[p][:, 1:2],
                                        scalar1=EPS, scalar2=None, op0=add)
                newton_rsqrt(ya[p], za[p], ta[p], SEED1)
                nc.vector.tensor_tensor(out=s1[p], in0=g1_sb[:, p:p + 1],
                                        in1=ya[p], op=mult)
                nc.vector.tensor_scalar(out=ta[p], in0=s1[p], scalar1=-1.0,
                                        scalar2=None, op0=mult)
                nc.vector.scalar_tensor_tensor(
                    out=t1[p], in0=mv[p][:, 0:1], scalar=ta[p],
                    in1=bt1_sb[:, p:p + 1], op0=mult, op1=add)

            def emit_B(p):
                for sc in range(NSQ):            # super-chunks of 2048
                    o1 = o1p.tile([D, SQCH], bf16, tag="o1")
                    for q in range(4):
                        ps = psb.tile([D, CH], f32, tag="B")
                        lo = sc * SQCH + q * CH
                        nc.tensor.matmul(ps, lhsT=wcol(w1_sb, p),
                                         rhs=xu[p][:, lo:lo + CH],
                                         start=True, stop=True)
                        nc.scalar.activation(out=o1[:, q * CH:(q + 1) * CH],
                                             in_=ps, func=Tanh,
                                             bias=t1[p], scale=s1[p])
                    for uq in range(2):          # u-chunks of 1024
                        h = sc * 2 + uq
                        on_act = h in ACT_U
                        pc = psc.tile([D, UCH], f32, tag="C")
                        for r in range(2):
                            nc.tensor.matmul(
                                pc[:, r * CH:(r + 1) * CH], lhsT=wcol(w2_sb, p),
                                rhs=o1[:, uq * UCH + r * CH:uq * UCH + (r + 1) * CH],
                                start=True, stop=not on_act)
                        us = xu[p][:, h * UCH:(h + 1) * UCH]
                        if on_act:
                            for r in range(2):
                                nc.tensor.matmul(
                                    pc[:, r * CH:(r + 1) * CH], lhsT=id_sb,
                                    rhs=us[:, r * CH:(r + 1) * CH],
                                    start=False, stop=True)
                            nc.scalar.activation(out=us, in_=pc, func=Copy,
                                                 accum_out=sumB[p][:, h:h + 1])
                        else:
                            nc.vector.scalar_tensor_tensor(
                                out=us, in0=pc, scalar=1.0, in1=us,
                                op0=mult, op1=add,
                                accum_out=sumB[p][:, h:h + 1])
                    scr = scp.tile([D, SQCH], bf16, tag="sq")
                    usq = xu[p][:, sc * SQCH:(sc + 1) * SQCH]
                    nc.vector.tensor_tensor(out=scr, in0=usq, in1=usq, op=mult)
                    nc.vector.tensor_scalar(out=scr, in0=scr, scalar1=1.0,
                                            scalar2=0.0, op0=mult, op1=add,
                                            accum_out=sqB[p][:, sc:sc + 1])
                # affine2 from (sum u, sum u^2)
                nc.vector.tensor_reduce(out=ms[p][:, 0:1], in_=sumB[p],
                                        axis=AX, op=add)
                nc.vector.tensor_reduce(out=ms[p][:, 1:2], in_=sqB[p],
                                        axis=AX, op=add)
                nc.vector.tensor_scalar(out=ms[p], in0=ms[p], scalar1=1.0 / R,
                                        scalar2=None, op0=mult)
                # za = -(mean^2 - msq) + eps = var + eps
                nc.vector.scalar_tensor_tensor(
                    out=za[p], in0=ms[p][:, 0:1], scalar=ms[p][:, 0:1],
                    in1=ms[p][:, 1:2], op0=mult, op1=subtract)
                nc.vector.tensor_scalar(out=za[p], in0=za[p], scalar1=-1.0,
                                        scalar2=EPS, op0=mult, op1=add)
                newton_rsqrt(ya[p], za[p], ta[p], SEED2)
                nc.vector.tensor_tensor(out=s3[p], in0=g3_sb[:, p:p + 1],
                                        in1=ya[p], op=mult)
                nc.vector.tensor_scalar(out=ta[p], in0=s3[p], scalar1=-1.0,
                                        scalar2=None, op0=mult)
                nc.vector.scalar_tensor_tensor(
                    out=t3[p], in0=ms[p][:, 0:1], scalar=ta[p],
                    in1=bt3_sb[:, p:p + 1], op0=mult, op1=add)

            def emit_C(p):
                for t in range(NTC):
                    ob = obp.tile([D, TCH], bf16, tag="ob")
                    nc.scalar.activation(out=ob,
                                         in_=xu[p][:, t * TCH:(t + 1) * TCH],
                                         func=Tanh, bias=t3[p], scale=s3[p])
                    nc.sync.dma_start(out[p * D:(p + 1) * D,
                                          t * TCH:(t + 1) * TCH], ob)

            # software pipeline over the 4 blocks
            emit_A(0)
            emit_A(1)
            emit_B(0)
            emit_A(2)
            emit_B(1)
            emit_C(0)
            emit_A(3)
            emit_B(2)
            emit_C(1)
            emit_B(3)
            emit_C(2)
            emit_C(3)

    return nc


def _get_nc():
    if "nc" not in _state:
        _install_tile_drain_patch()
        _install_ldw_opt_patch()
        _install_ntff_hook()
        _state["nc"] = _build()
    return _state["nc"]


def kernel(x, weights1, bias1, weights2, bias2, gamma1, beta1, gamma3, beta3):
    from concourse.bass_utils import run_bass_kernel_spmd

    x = np.asarray(x, dtype=np.float32)
    w1 = np.asarray(weights1, dtype=np.float32)
    w2 = np.asarray(weights2, dtype=np.float32)
    gamma1 = np.asarray(gamma1, dtype=np.float32).reshape(P, D)
    beta1 = np.asarray(beta1, dtype=np.float32).reshape(P, D)
    gamma3 = np.asarray(gamma3, dtype=np.float32).reshape(P, D)
    beta3 = np.asarray(beta3, dtype=np.float32).reshape(P, D)

    nc = _get_nc()

    xT = np.ascontiguousarray(x.T).astype(_BF16)            # [F, B]
    identh = np.eye(D, dtype=np.float32).astype(_BF16)

    in_maps = []
    for cid in range(NCORES):
        blocks = list(range(cid * NBLK, (cid + 1) * NBLK))
        w1h = np.ascontiguousarray(np.concatenate([w1[p] for p in blocks], axis=1)).astype(_BF16)
        w2h = np.ascontiguousarray(np.concatenate([w2[p] for p in blocks], axis=1)).astype(_BF16)
        in_maps.append({
            "xt": np.ascontiguousarray(xT[cid * FC:(cid + 1) * FC, :]),
            "w1": w1h, "w2": w2h, "ident": identh,
            "g1": np.ascontiguousarray(gamma1[blocks].T),
            "bt1": np.ascontiguousarray(beta1[blocks].T),
            "g3": np.ascontiguousarray(gamma3[blocks].T),
            "bt3": np.ascontiguousarray(beta3[blocks].T),
        })

    res = run_bass_kernel_spmd(nc, in_maps, core_ids=list(range(NCORES)))
    _state["last_exec_time_ns"] = res.exec_time_ns

    outF = np.empty((B, F), dtype=np.float32)
    for cid in range(NCORES):
        outF[:, cid * FC:(cid + 1) * FC] = res.results[cid]["out"].T.astype(np.float32)
    return outF


# revision 16
# speedup vs baseline: 1.2181x; 1.0510x over previous
"""Trainium2 Bass kernel for nn_Better_Transformer (block-diag MLP + BatchNorm + tanh ×2).

  o1 = tanh(BN(x @ blockdiag(w1) + b1))
  o3 = tanh(BN(o1 @ blockdiag(w2) + b2 + x))

Strategy (8 NeuronCores, FEATURE-sharded — zero collectives):
  The network is fully block-diagonal per 128-feature block: output block p
  depends only on input block p (block matmuls are per-block, the residual is
  elementwise, BatchNorm normalizes over the batch dim).  Each core owns 4 of
  the 32 blocks (512 features) and ALL 16384 rows, so BN statistics are fully
  core-local and both AllReduce sync points of the data-parallel layout
  disappear.

  Per core, feature-major layout ([128 features, 16384 rows] bf16 per block):
  - Stage A (per block): matmul1 chunks -> PSUM, bn_stats on VectorE.
    bias1/bias2 cancel inside BatchNorm and never reach the device.
  - rstd via Newton iteration on VectorE (no Sqrt on ScalarE -> the ACT
    engine runs a single act-table set {Tanh, Copy} with zero reloads).
  - Stage B (per block): matmul1 recomputed (cheaper than storing y1),
    tanh1 on ScalarE (PSUM->SBUF bf16), matmul2, then u = o2 + x:
    ~11/16 of chunks add x via an identity matmul on the TensorEngine and
    copy PSUM->SBUF on ScalarE (Copy + accum_out gives sum(u) free);
    ~5/16 of chunks use a fused scalar_tensor_tensor on VectorE
    (add + copy + sum in one op).  u overwrites x in place.
    sum(u^2) via tensor_tensor square (2x bf16 mode) + tensor_scalar accum
    (4x mode) on VectorE.
  - Stage C (per block): tanh2 on ScalarE (SBUF->SBUF), DMA out.
  - Blocks are software-pipelined (A0 A1 B0 A2 B1 C0 A3 B2 C1 B3 C2 C3) so
    each engine's in-order queue never convoys on another engine.
  - A warm-up burst of matmuls trips the PE HAM throttle up while the input
    DMAs are still in flight.
"""

import os
import sys
import types

import numpy as np
import ml_dtypes

B, F, P, D = 16384, 4096, 32, 128
NCORES = 8
NBLK = P // NCORES        # 4 blocks per core
FC = NBLK * D             # 512 features per core
R = B                     # all 16384 rows on every core
CH = 512                  # stage-A chunk / matmul moving width
NCA = R // CH             # 32 stats chunks per block
UCH = 1024                # u-production chunk
NCU = R // UCH            # 16 u chunks per block
SQCH = 2048               # squared-sum chunk
NSQ = R // SQCH           # 8
TCH = 4096                # tanh2 chunk
NTC = R // TCH            # 4
EPS = 1e-5

# u-production engine split: chunks in DVE_U run on VectorE (fused STT),
# the rest on ScalarE (identity-matmul residual + Copy w/ accum).
DVE_U = (0, 2, 4, 6, 8, 10, 12, 14)
ACT_U = tuple(h for h in range(NCU) if h not in DVE_U)

MM_FD1024 = False         # FD1024 matmuls fail the walrus ISA check (1 bank max)
POOL_REDUCE = False       # GpSimd fails the walrus engine check for TensorScalarPtr
POOL_AFFINE = False       # same: affine chains must stay on VectorE

NEWTON_ITERS = 5
SEED1 = 1.7               # 1/sqrt(var(y1)+eps), var(y1) ~ 1/3
SEED2 = 0.9               # 1/sqrt(var(u)+eps),  var(u)  ~ 1.2

_BF16 = ml_dtypes.bfloat16

_state: dict = {}


def _install_ldw_opt_patch():
    """bass hardcodes --enable-ldw-opt=false; walrus's own default is
    true.  Re-enable it (BASS_LDW_OPT=0 reverts) so repeated-lhsT matmul
    runs don't reload the PE weight array every instruction."""
    if _state.get("ldw_patched") or os.environ.get("BASS_LDW_OPT", "0") != "1":
        return
    _state["ldw_patched"] = True
    import concourse.bass_utils as bu
    real = bu.run_command

    def wrapper(argv, **kw):
        argv = ["--enable-ldw-opt=true" if a == "--enable-ldw-opt=false" else a
                for a in argv]
        return real(argv, **kw)

    bu.run_command = wrapper


def _install_tile_drain_patch():
    """This walrus build rejects >1 sem wait per instruction ("Too many
    sync wait commands" in setupSyncWait).  1) split the end-of-kernel
    drain waits across single-wait NOPs; 2) after assign_waits, hoist
    extra per-instruction waits onto nofuse NOPs."""
    if _state.get("patched"):
        return
    _state["patched"] = True
    import concourse.mybir as mybir
    import concourse.tile as tile_mod
    from concourse.tile import TileContext
    from concourse.vector_clock import ScopedClock, VectorClock

    def _drain_and_barrier(self, tick_clock, wait_clock):
        gc = tick_clock.global_clock
        for i in range(len(gc)):
            if gc[i] > 0:
                c = VectorClock()
                c.require_at_least(i, gc[i])
                nop = self.nc.sync.nop(nofuse=True, hint="tile_exit_wait")
                wait_clock.add_sem_waits(nop.ins, ScopedClock({None: c}))
        self.nc.sync.drain()
        self.nc.all_engine_barrier()
        assert self.sems is not None
        popped = self.nc._tile_sem_poison_stack.pop()
        assert popped is self._sem_poison
        self.nc.clear_and_free_semaphores(list(self.sems.allocated().values()))
        self.nc.all_engine_barrier()

    TileContext._drain_and_barrier = _drain_and_barrier

    _RealWait = tile_mod.TileClockWait

    class _WaitSplitClockWait:
        def __init__(self, tc, ordered):
            self._w = _RealWait(tc, ordered)
            self._tc = tc
            self._ordered = ordered

        def assign_waits(self, bb_name):
            r = self._w.assign_waits(bb_name)
            nc = self._tc.nc
            for insts in self._ordered.values():
                out = []
                for inst in insts:
                    si = inst.sync_info
                    if si is not None and si.on_wait and len(si.on_wait) > 1:
                        waits = list(si.on_wait)
                        for w in waits[:-1]:
                            nop = mybir.InstNoOp(
                                name=nc.get_next_instruction_name(),
                                engine=inst.engine, ins=[], outs=[],
                            )
                            nop.bass_nofuse = True
                            nop.sync_info = mybir.SyncInfo(on_wait=[w], on_update=[])
                            out.append(nop)
                        si.on_wait = [waits[-1]]
                    out.append(inst)
                insts[:] = out
            return r

        def __getattr__(self, k):
            return getattr(self._w, k)

    tile_mod.TileClockWait = _WaitSplitClockWait


def _install_ntff_hook():
    """Optional: lets BASS_TRACE=1 produce an NTFF profile under axon when
    the image's antenv lacks axon_hooks.  Safe no-op on any failure."""
    if "antenv.axon_hooks" in sys.modules:
        return
    try:
        import contextlib
        import ctypes

        so_path = "/opt/axon/libaxon_pjrt.so"
        if not os.path.exists(so_path):
            return
        lib = ctypes.CDLL(so_path)
        if not hasattr(lib, "axon_start_nrt_profile"):
            return
        lib.axon_start_nrt_profile.argtypes = [ctypes.POINTER(ctypes.c_int64), ctypes.c_size_t]
        lib.axon_start_nrt_profile.restype = ctypes.c_int64
        lib.axon_stop_nrt_profile.argtypes = [ctypes.c_char_p]
        lib.axon_stop_nrt_profile.restype = ctypes.c_int64

        @contextlib.contextmanager
        def _hook(output_dir, device_ids):
            import jax
            jax.devices()
            if device_ids:
                ids = (ctypes.c_int64 * len(device_ids))(*device_ids)
                rc = lib.axon_start_nrt_profile(ids, len(device_ids))
            else:
                rc = lib.axon_start_nrt_profile(None, 0)
            if rc != 0:
                raise RuntimeError(f"axon_start_nrt_profile rc={rc}")
            try:
                yield
            finally:
                n = lib.axon_stop_nrt_profile(str(output_dir).encode())
                if n <= 0:
                    print(f"ntff profile: {n} files written", file=sys.stderr)

        mod = types.ModuleType("antenv.axon_hooks")
        mod.get_axon_ntff_profile_hook = lambda: _hook
        mod.set_axon_ntff_profile_hook = lambda h: None
        sys.modules["antenv.axon_hooks"] = mod
    except Exception:
        pass


def _build():
    import concourse.bass as bass
    import concourse.mybir as mybir
    import concourse.tile as tile

    f32 = mybir.dt.float32
    bf16 = mybir.dt.bfloat16
    Tanh = mybir.ActivationFunctionType.Tanh
    Copy = mybir.ActivationFunctionType.Copy
    mult = mybir.AluOpType.mult
    add = mybir.AluOpType.add
    subtract = mybir.AluOpType.subtract
    AX = mybir.AxisListType.X

    nc = bass.Bass(trn_type="TRN2", num_devices=NCORES)

    xt = nc.dram_tensor("xt", [FC, R], bf16, kind="ExternalInput")
    w1 = nc.dram_tensor("w1", [D, FC], bf16, kind="ExternalInput")
    w2 = nc.dram_tensor("w2", [D, FC], bf16, kind="ExternalInput")
    ident = nc.dram_tensor("ident", [D, D], bf16, kind="ExternalInput")
    g1 = nc.dram_tensor("g1", [D, NBLK], f32, kind="ExternalInput")
    bt1 = nc.dram_tensor("bt1", [D, NBLK], f32, kind="ExternalInput")
    g3 = nc.dram_tensor("g3", [D, NBLK], f32, kind="ExternalInput")
    bt3 = nc.dram_tensor("bt3", [D, NBLK], f32, kind="ExternalInput")
    out = nc.dram_tensor("out", [FC, R], bf16, kind="ExternalOutput")

    with tile.TileContext(nc) as tc:
        with (
            tc.tile_pool(name="const", bufs=1) as const,
            tc.tile_pool(name="xup", bufs=1) as xup,
            tc.tile_pool(name="stat", bufs=1) as statp,
            tc.tile_pool(name="o1p", bufs=2) as o1p,
            tc.tile_pool(name="scp", bufs=2) as scp,
            tc.tile_pool(name="obp", bufs=2) as obp,
            tc.tile_pool(name="psb", bufs=2, space="PSUM") as psb,
            tc.tile_pool(name="psc", bufs=2, space="PSUM") as psc,
        ):
            w1_sb = const.tile([D, FC], bf16)
            w2_sb = const.tile([D, FC], bf16)
            id_sb = const.tile([D, D], bf16)
            g1_sb = const.tile([D, NBLK], f32)
            bt1_sb = const.tile([D, NBLK], f32)
            g3_sb = const.tile([D, NBLK], f32)
            bt3_sb = const.tile([D, NBLK], f32)
            nc.sync.dma_start(w1_sb, w1[:])
            nc.sync.dma_start(w2_sb, w2[:])
            nc.sync.dma_start(id_sb, ident[:])
            nc.sync.dma_start(g1_sb, g1[:])
            nc.sync.dma_start(bt1_sb, bt1[:])
            nc.sync.dma_start(g3_sb, g3[:])
            nc.sync.dma_start(bt3_sb, bt3[:])

            # PE HAM warm-up while the x DMAs stream in.
            for _ in range(24):
                pw = psc.tile([D, UCH], f32, tag="C")
                nc.tensor.matmul(pw[:, 0:CH], lhsT=w1_sb[:, 0:D],
                                 rhs=w1_sb[:, 0:CH], start=True, stop=True)

            xu = []
            for p in range(NBLK):
                t = xup.tile([D, R], bf16, tag=f"xu{p}")
                for q in range(4):
                    nc.sync.dma_start(t[:, q * (R // 4):(q + 1) * (R // 4)],
                                      xt[p * D:(p + 1) * D,
                                         q * (R // 4):(q + 1) * (R // 4)])
                xu.append(t)

            # per-block stat/state tiles (static: blocks are pipelined)
            st1 = [statp.tile([D, NCA, 6], f32, name=f"st1_{p}") for p in range(NBLK)]
            mv = [statp.tile([D, 2], f32, name=f"mv_{p}") for p in range(NBLK)]
            sumB = [statp.tile([D, NCU], f32, name=f"sumB_{p}") for p in range(NBLK)]
            sqB = [statp.tile([D, NSQ], f32, name=f"sqB_{p}") for p in range(NBLK)]
            s1 = [statp.tile([D, 1], f32, name=f"s1_{p}") for p in range(NBLK)]
            t1 = [statp.tile([D, 1], f32, name=f"t1_{p}") for p in range(NBLK)]
            s3 = [statp.tile([D, 1], f32, name=f"s3_{p}") for p in range(NBLK)]
            t3 = [statp.tile([D, 1], f32, name=f"t3_{p}") for p in range(NBLK)]
            za = [statp.tile([D, 1], f32, name=f"za_{p}") for p in range(NBLK)]
            ya = [statp.tile([D, 1], f32, name=f"ya_{p}") for p in range(NBLK)]
            ta = [statp.tile([D, 1], f32, name=f"ta_{p}") for p in range(NBLK)]
            ms = [statp.tile([D, 2], f32, name=f"ms_{p}") for p in range(NBLK)]

            def wcol(w_sb, p):
                return w_sb[:, p * D:(p + 1) * D]

            ae = nc.gpsimd if POOL_AFFINE else nc.vector

            def mm_wide(pt, lhsT, rhs_lo, on_act=None):
                # fill a [D, UCH] psum tile from rhs columns [lo, lo+UCH)
                if MM_FD1024:
                    nc.tensor.matmul(pt, lhsT=lhsT, rhs=rhs_lo,
                                     start=True, stop=True)
                else:
                    for r in range(2):
                        nc.tensor.matmul(pt[:, r * CH:(r + 1) * CH], lhsT=lhsT,
                                         rhs=rhs_lo[:, r * CH:(r + 1) * CH],
                                         start=True, stop=True)

            def newton_rsqrt(y, z, tmp, seed):
                # y := 1/sqrt(z), z > 0
                ae.memset(y, seed)
                for _ in range(NEWTON_ITERS):
                    ae.scalar_tensor_tensor(
                        out=tmp, in0=y, scalar=y, in1=z, op0=mult, op1=mult)
                    ae.tensor_scalar(
                        out=tmp, in0=tmp, scalar1=-0.5, scalar2=1.5,
                        op0=mult, op1=add)
                    ae.tensor_scalar(
                        out=y, in0=y, scalar1=tmp, scalar2=None, op0=mult)

            def emit_A(p):
                for h in range(NCU):
                    ps = psc.tile([D, UCH], f32, tag="C")
                    mm_wide(ps, wcol(w1_sb, p), xu[p][:, h * UCH:(h + 1) * UCH])
                    nc.vector.bn_stats(out=st1[p][:, 2 * h], in_=ps[:, 0:CH])
                    nc.vector.bn_stats(out=st1[p][:, 2 * h + 1], in_=ps[:, CH:UCH])
                nc.vector.bn_aggr(out=mv[p], in_=st1[p])
                # affine1: s1 = g1 * rstd ; t1 = b1 - mean * s1
                ae.tensor_scalar(out=za[p], in0=mv[p][:, 1:2],
                                 scalar1=EPS, scalar2=None, op0=add)
                newton_rsqrt(ya[p], za[p], ta[p], SEED1)
                ae.tensor_tensor(out=s1[p], in0=g1_sb[:, p:p + 1],
                                 in1=ya[p], op=mult)
                ae.tensor_scalar(out=ta[p], in0=s1[p], scalar1=-1.0,
                                 scalar2=None, op0=mult)
                ae.scalar_tensor_tensor(
                    out=t1[p], in0=mv[p][:, 0:1], scalar=ta[p],
                    in1=bt1_sb[:, p:p + 1], op0=mult, op1=add)

            def emit_B(p):
                for sc in range(NSQ):            # super-chunks of 2048
                    o1 = o1p.tile([D, SQCH], bf16, tag="o1")
                    for q in range(2):
                        ps = psb.tile([D, UCH], f32, tag="B")
                        lo = sc * SQCH + q * UCH
                        mm_wide(ps, wcol(w1_sb, p), xu[p][:, lo:lo + UCH])
                        nc.scalar.activation(out=o1[:, q * UCH:(q + 1) * UCH],
                                             in_=ps, func=Tanh,
                                             bias=t1[p], scale=s1[p])
                    for uq in range(2):          # u-chunks of 1024
                        h = sc * 2 + uq
                        on_act = h in ACT_U
                        pc = psc.tile([D, UCH], f32, tag="C")
                        if MM_FD1024:
                            nc.tensor.matmul(
                                pc, lhsT=wcol(w2_sb, p),
                                rhs=o1[:, uq * UCH:(uq + 1) * UCH],
                                start=True, stop=not on_act)
                        else:
                            for r in range(2):
                                nc.tensor.matmul(
                                    pc[:, r * CH:(r + 1) * CH], lhsT=wcol(w2_sb, p),
                                    rhs=o1[:, uq * UCH + r * CH:uq * UCH + (r + 1) * CH],
                                    start=True, stop=not on_act)
                        us = xu[p][:, h * UCH:(h + 1) * UCH]
                        if on_act:
                            if MM_FD1024:
                                nc.tensor.matmul(pc, lhsT=id_sb, rhs=us,
                                                 start=False, stop=True)
                            else:
                                for r in range(2):
                                    nc.tensor.matmul(
                                        pc[:, r * CH:(r + 1) * CH], lhsT=id_sb,
                                        rhs=us[:, r * CH:(r + 1) * CH],
                                        start=False, stop=True)
                            nc.scalar.activation(out=us, in_=pc, func=Copy,
                                                 accum_out=sumB[p][:, h:h + 1])
                        else:
                            nc.vector.scalar_tensor_tensor(
                                out=us, in0=pc, scalar=1.0, in1=us,
                                op0=mult, op1=add,
                                accum_out=sumB[p][:, h:h + 1])
                    scr = scp.tile([D, SQCH], bf16, tag="sq")
                    usq = xu[p][:, sc * SQCH:(sc + 1) * SQCH]
                    if POOL_REDUCE:
                        nc.gpsimd.scalar_tensor_tensor(
                            out=scr, in0=usq, scalar=1.0, in1=usq,
                            op0=mult, op1=mult,
                            accum_out=sqB[p][:, sc:sc + 1])
                    else:
                        nc.vector.scalar_tensor_tensor(
                            out=scr, in0=usq, scalar=1.0, in1=usq,
                            op0=mult, op1=mult,
                            accum_out=sqB[p][:, sc:sc + 1])
                # affine2 from (sum u, sum u^2); free-axis reduce is DVE-only
                nc.vector.tensor_reduce(out=ms[p][:, 0:1], in_=sumB[p],
                                        axis=AX, op=add)
                nc.vector.tensor_reduce(out=ms[p][:, 1:2], in_=sqB[p],
                                        axis=AX, op=add)
                ae.tensor_scalar(out=ms[p], in0=ms[p], scalar1=1.0 / R,
                                 scalar2=None, op0=mult)
                # za = -(mean^2 - msq) + eps = var + eps
                ae.scalar_tensor_tensor(
                    out=za[p], in0=ms[p][:, 0:1], scalar=ms[p][:, 0:1],
                    in1=ms[p][:, 1:2], op0=mult, op1=subtract)
                ae.tensor_scalar(out=za[p], in0=za[p], scalar1=-1.0,
                                 scalar2=EPS, op0=mult, op1=add)
                newton_rsqrt(ya[p], za[p], ta[p], SEED2)
                ae.tensor_tensor(out=s3[p], in0=g3_sb[:, p:p + 1],
                                 in1=ya[p], op=mult)
                ae.tensor_scalar(out=ta[p], in0=s3[p], scalar1=-1.0,
                                 scalar2=None, op0=mult)
                ae.scalar_tensor_tensor(
                    out=t3[p], in0=ms[p][:, 0:1], scalar=ta[p],
                    in1=bt3_sb[:, p:p + 1], op0=mult, op1=add)

            def emit_C(p):
                for t in range(NTC):
                    ob = obp.tile([D, TCH], bf16, tag="ob")
                    nc.scalar.activation(out=ob,
                                         in_=xu[p][:, t * TCH:(t + 1) * TCH],
                                         func=Tanh, bias=t3[p], scale=s3[p])
                    nc.sync.dma_start(out[p * D:(p + 1) * D,
                                          t * TCH:(t + 1) * TCH], ob)

            # software pipeline over the 4 blocks
            emit_A(0)
            emit_A(1)
            emit_B(0)
            emit_C(0)
            emit_A(2)
            emit_B(1)
            emit_C(1)
            emit_A(3)
            emit_B(2)
            emit_C(2)
            emit_B(3)
            emit_C(3)

    return nc


def _get_nc():
    if "nc" not in _state:
        _install_tile_drain_patch()
        _install_ldw_opt_patch()
        _install_ntff_hook()
        _state["nc"] = _build()
    return _state["nc"]


def kernel(x, weights1, bias1, weights2, bias2, gamma1, beta1, gamma3, beta3):
    from concourse.bass_utils import run_bass_kernel_spmd

    x = np.asarray(x, dtype=np.float32)
    w1 = np.asarray(weights1, dtype=np.float32)
    w2 = np.asarray(weights2, dtype=np.float32)
    gamma1 = np.asarray(gamma1, dtype=np.float32).reshape(P, D)
    beta1 = np.asarray(beta1, dtype=np.float32).reshape(P, D)
    gamma3 = np.asarray(gamma3, dtype=np.float32).reshape(P, D)
    beta3 = np.asarray(beta3, dtype=np.float32).reshape(P, D)

    nc = _get_nc()

    xT = np.ascontiguousarray(x.T).astype(_BF16)            # [F, B]
    identh = np.eye(D, dtype=np.float32).astype(_BF16)

    in_maps = []
    for cid in range(NCORES):
        blocks = list(range(cid * NBLK, (cid + 1) * NBLK))
        w1h = np.ascontiguousarray(np.concatenate([w1[p] for p in blocks], axis=1)).astype(_BF16)
        w2h = np.ascontiguousarray(np.concatenate([w2[p] for p in blocks], axis=1)).astype(_BF16)
        in_maps.append({
            "xt": np.ascontiguousarray(xT[cid * FC:(cid + 1) * FC, :]),
            "w1": w1h, "w2": w2h, "ident": identh,
            "g1": np.ascontiguousarray(gamma1[blocks].T),
            "bt1": np.ascontiguousarray(beta1[blocks].T),
            "g3": np.ascontiguousarray(gamma3[blocks].T),
            "bt3": np.ascontiguousarray(beta3[blocks].T),
        })

    res = run_bass_kernel_spmd(nc, in_maps, core_ids=list(range(NCORES)))
    _state["last_exec_time_ns"] = res.exec_time_ns

    outF = np.empty((B, F), dtype=np.float32)
    for cid in range(NCORES):
        outF[:, cid * FC:(cid + 1) * FC] = res.results[cid]["out"].T.astype(np.float32)
    return outF
